# revision 45
# baseline (speedup 1.0000x reference)
"""Multi-head attention (B=2, S=4096, D=512, H=8, DR=64) on 8 trn2 NeuronCores.

Sharding: core c -> batch b = c // 4, head-pair hp = c % 4 (heads 2*hp, 2*hp+1).
Each core computes q/k/v projections, flash-style attention with scores kept
in transposed [t, s] orientation, and the partial output projection for its
two heads. Host sums the 4 partials per batch and adds the bias.

Pipeline notes (all on-chip data bf16; fp32 only in PSUM accumulators):
- exp alternates whole iterations between ScalarE (exact exp) and VectorE
  (Schraudolph fast-exp: int16(x*A+B) bit-cast as bf16). Alternating whole
  iterations (rather than splitting columns of one tile) keeps the two
  engines' writes on disjoint tiles, so Tile's hazard tracking lets them
  run concurrently; the serial exp stream was the original bottleneck.
  The softmax denominator sums the same quantized weights, so attention
  rows still sum to 1; the fast-exp error averages out over 4096-key sums.
- AV keeps the ones-column trick (M=65 per head): softmax denominators
  accumulate for free in row 64 of each po bank.
- Normalization is deferred past the output projection: y = py0*r0 + py1*r1
  with per-partition reciprocals obtained by round-tripping the Z rows
  through DRAM (DMA moves data across partitions; engines cannot).
"""

import sys

for _p in ("/opt/trn_rl_repo", "/root/.axon_site/_ro/trn_rl_repo"):
    if _p not in sys.path:
        sys.path.insert(0, _p)

import os as _osmod
import numpy as np
from contextlib import ExitStack

KDBG = _osmod.environ.get("KDBG", "")

import concourse.bass as bass
import concourse.tile as tile
import concourse.mybir as mybir
from concourse.bass_utils import run_bass_kernel_spmd
from concourse.masks import make_identity

B, S, D = 2, 4096, 512
H, DR = 8, 64
P = 128
NT = S // P          # 32 t-tiles (also s-tiles)
SBW = 512            # s-block width
NSB = S // SBW       # 8 s-blocks / t-groups
DC = D // P          # 4 d-chunks
GT = SBW // P        # 4 t-tiles per group
N_CORES = 8
FP32 = mybir.dt.float32
BF16 = mybir.dt.bfloat16
I16 = mybir.dt.int16

# exp split and fast-exp constants (bf16 bit-trick domain)
CEXP = 352           # ScalarE handles cols [0:CEXP) per head per 512-block
LOG2E = 1.4426950408889634
A16 = 128.0 * LOG2E / 8.0           # applied to raw (unscaled) scores
B16 = 128.0 * (127.0 - 0.0861)


# This repo's walrus invocation hardcodes --enable-ldw-opt=false, which keeps
# every LDWEIGHTS serialized with its MATMUL. walrus' LDW optimization rejects
# standalone InstLdweights, so fuse each one into its following Matmult
# (ldweights=true) in the BIR json, then flip the flag on the command line.
def _fuse_ldweights_json(path_in, path_out):
    import json as _json

    d = _json.load(open(path_in))
    n_noop = [0]
    for f in d.get("functions", []):
        for b in f.get("blocks", []):
            insts = b.get("instructions", [])
            out = []
            pend = None
            for i in insts:
                if i.get("engine") != "PE":
                    out.append(i)
                    continue
                if i["opcode"] == "Ldweights":
                    assert pend is None
                    pend = i
                    continue
                if i["opcode"] == "Matmult" and pend is not None:
                    assert pend["ins"][0] == i["ins"][1], (
                        f"weights AP mismatch {pend['name']} vs {i['name']}"
                    )
                    i["ldweights"] = True
                    sl = pend.get("sync_info") or {}
                    sm = i.get("sync_info") or {}
                    waits = list(sl.get("on_wait") or []) + list(
                        sm.get("on_wait") or []
                    )
                    upds = list(sl.get("on_update") or []) + list(
                        sm.get("on_update") or []
                    )
                    while len(waits) > 1:
                        w = waits.pop(0)
                        n_noop[0] += 1
                        out.append(
                            {
                                "name": f"I-fusenoop-{n_noop[0]}",
                                "engine": "PE",
                                "opcode": "NoOp",
                                "debug": i.get("debug", 0),
                                "ins": [],
                                "outs": [],
                                "sync_info": {"on_wait": [w], "on_update": []},
                            }
                        )
                    i["sync_info"] = {"on_wait": waits, "on_update": upds}
                    pend = None
                    out.append(i)
                    continue
                out.append(i)
            assert pend is None, "trailing Ldweights"
            b["instructions"] = out
    _json.dump(d, open(path_out, "w"))


def _patch_ldw_opt():
    import os as _os
    from concourse import bass_utils as _bu

    if getattr(_bu, "_ldw_opt_patched", False):
        return
    _bu._ldw_opt_patched = True
    _orig = _bu.run_command

    def patched_run(argv, **kwargs):
        return _orig(argv, **kwargs)

    _bu.run_command = patched_run


_patch_ldw_opt()

_drain_patched = False


def _patch_tile_drain():
    """This walrus build rejects >1 sync wait on one instruction, which breaks
    TileContext's kernel-tail drain. Spread the waits over nop instructions
    emitted just before the drain."""
    global _drain_patched
    if _drain_patched:
        return
    _drain_patched = True

    def patched(self, tick_clock, wait_clock):
        nop0 = self.nc.sync.nop()
        wait_clock.add_sem_waits(
            nop0.ins, tile.ScopedClock({None: tick_clock.global_clock})
        )
        si = nop0.ins.sync_info
        waits = list(si.on_wait) if si is not None else []
        if waits:
            nop0.ins.sync_info = mybir.SyncInfo(on_wait=waits[:1], on_update=[])
            for w in waits[1:]:
                nop = self.nc.sync.nop()
                nop.ins.sync_info = mybir.SyncInfo(on_wait=[w], on_update=[])
        self.nc.sync.drain()
        self.nc.all_engine_barrier()
        popped = self.nc._tile_sem_poison_stack.pop()
        assert popped is self._sem_poison
        self.nc.clear_and_free_semaphores(list(self.sems.allocated().values()))
        self.nc.all_engine_barrier()

    tile.TileContext._drain_and_barrier = patched


# This walrus build supports only one sync-wait slot per instruction, while
# Tile's sem-assigner attaches up to ~3. Spread the excess onto NoOp
# instructions inserted immediately before the owning instruction.
_WAIT_LIMIT = 1
_SKIP_OPCODES = {"AllEngineBarrier", "EventSemaphore", "Call"}


def _split_sync_waits(nc: bass.Bass):
    noop_cls = getattr(mybir, "InstNoOp", None)
    if noop_cls is None:
        import bass_rust

        noop_cls = bass_rust.InstNoOp
    counter = [0]
    for f in nc.m.functions:
        for blk in f.blocks:
            insts = blk.instructions
            new_list = []
            changed = False
            for inst in insts:
                si = inst.sync_info
                waits = list(si.on_wait) if si is not None and si.on_wait else []
                wait_limit = 0 if inst.opcode == "Ldweights" else _WAIT_LIMIT
                if (
                    len(waits) > wait_limit
                    and inst.opcode not in _SKIP_OPCODES
                    and all(w.sync_type == "semaphore" for w in waits)
                ):
                    excess = waits[: len(waits) - wait_limit]
                    keep = waits[len(waits) - wait_limit :]
                    for w in excess:
                        counter[0] += 1
                        new_list.append(
                            noop_cls(
                                name=f"I-waitsplit-{counter[0]}",
                                engine=inst.engine,
                                debug=inst.debug,
                                ins=[],
                                outs=[],
                                sync_info=mybir.SyncInfo(
                                    on_wait=[w], on_update=[]
                                ),
                            )
                        )
                    inst.sync_info = mybir.SyncInfo(
                        on_wait=keep, on_update=list(si.on_update or [])
                    )
                    changed = True
                new_list.append(inst)
            if changed:
                insts.clear()
                insts.extend(new_list)


def _build_program() -> bass.Bass:
    _patch_tile_drain()
    nc = bass.Bass()

    xt_d = nc.declare_dram_parameter("xt", [D, S], BF16, isOutput=False)
    wq_d = nc.declare_dram_parameter("wq", [D, P], BF16, isOutput=False)
    wk_d = nc.declare_dram_parameter("wk", [D, P], BF16, isOutput=False)
    wv_d = nc.declare_dram_parameter("wv", [D, P], BF16, isOutput=False)
    wo_d = nc.declare_dram_parameter("wo", [P, D], BF16, isOutput=False)
    y_d = nc.declare_dram_parameter("y", [S, D], FP32, isOutput=True)
    zscr_d = nc.declare_dram_parameter("zscr", [NSB, 2, SBW], FP32, isOutput=True)

    with tile.TileContext(nc) as tc, ExitStack() as ctx:
        consts = ctx.enter_context(tc.tile_pool(name="consts", bufs=1))
        wpool = ctx.enter_context(tc.tile_pool(name="weights", bufs=1))
        aux = ctx.enter_context(tc.tile_pool(name="aux", bufs=2, space="PSUM"))
        psp = ctx.enter_context(tc.tile_pool(name="ps", bufs=2, space="PSUM"))
        pop = ctx.enter_context(tc.tile_pool(name="po", bufs=2, space="PSUM"))
        epool = ctx.enter_context(tc.tile_pool(name="exp", bufs=12))
        spool = ctx.enter_context(tc.tile_pool(name="small", bufs=4))
        opool = ctx.enter_context(tc.tile_pool(name="osb", bufs=2))
        ypool = ctx.enter_context(tc.tile_pool(name="yout", bufs=3))


        # PE warm-up: junk matmuls during the initial DMA window keep the HAM
        # clock-gate at 8/8 so the first real matmuls run at 2.4 GHz.
        warm = consts.tile([P, D], BF16)
        nc.vector.memset(warm[:], 0.0)
        pw = aux.tile([P, D], FP32, tag="aux", name="pw")
        for _ in range(10):
            nc.tensor.matmul(
                pw[:], warm[:, 0:P], warm[:], start=True, stop=True
            )
        if "selprobe" in KDBG:
            pdbg = aux.tile([P, 16], FP32, tag="aux", name="pdbg")
            nc.tensor.matmul(
                pdbg[:, 0:4], warm[:, 0:P], zsel[:], start=True, stop=True
            )
            sdbg = spool.tile([P, 4], FP32, tag="sdbg")
            nc.vector.tensor_copy(sdbg[:], pdbg[:, 0:4])

        # Weights fp16; w*_b[p, c*128 + e] = W[c*128 + p, e]
        wq_b = wpool.tile([P, D], BF16)
        wk_b = wpool.tile([P, D], BF16)
        wv_b = wpool.tile([P, D], BF16)
        wo_b = wpool.tile([P, D], BF16)
        for w_b, w_dram in ((wq_b, wq_d), (wk_b, wk_d), (wv_b, wv_d)):
            nc.gpsimd.dma_start(
                w_b[:].rearrange("p (c e) -> p c e", c=DC),
                w_dram[:].rearrange("(c p) e -> p c e", p=P),
            )
        nc.gpsimd.dma_start(wo_b[:], wo_d[:])

        # Per-group persistent tiles.
        # xT_g[g][p, c*512 + j] = x[g*512 + j, c*128 + p]
        xtp = ctx.enter_context(tc.tile_pool(name="xtg", bufs=NSB))
        ktp = ctx.enter_context(tc.tile_pool(name="ktg", bufs=NSB))
        qtp = ctx.enter_context(tc.tile_pool(name="qtg", bufs=NSB))
        vsp = ctx.enter_context(tc.tile_pool(name="vsg", bufs=NSB))
        xT_g = [None] * NSB
        kT_g = [None] * NSB   # [e(h0|h1), 512 t-cols]
        qT_g = [None] * NSB   # [e(h0|h1), 512 s-cols]
        v_g = [None] * NSB    # [t, 4 t-tiles * (h0 64 | h1 64)]

        def produce_x_q(g):
            xt = xtp.tile([P, DC * SBW], BF16, tag="xt")
            xT_g[g] = xt
            for c in range(DC):
                nc.sync.dma_start(
                    xt[:, c * SBW : (c + 1) * SBW],
                    xt_d[c * P : (c + 1) * P, g * SBW : (g + 1) * SBW],
                )
            qt = qtp.tile([P, SBW], BF16, tag="qt")
            qT_g[g] = qt
            pp = aux.tile([P, SBW], FP32, tag="aux")
            for c in range(DC):
                nc.tensor.matmul(
                    pp[:],
                    wq_b[:, c * P : (c + 1) * P],
                    xt[:, c * SBW : (c + 1) * SBW],
                    start=(c == 0),
                    stop=(c == DC - 1),
                )
            nc.vector.tensor_copy(qt[:], pp[:])

        def produce_k(g):
            xt = xT_g[g]
            kt = ktp.tile([P, SBW], BF16, tag="kt")
            kT_g[g] = kt
            pp = aux.tile([P, SBW], FP32, tag="aux")
            for c in range(DC):
                nc.tensor.matmul(
                    pp[:],
                    wk_b[:, c * P : (c + 1) * P],
                    xt[:, c * SBW : (c + 1) * SBW],
                    start=(c == 0),
                    stop=(c == DC - 1),
                )
            nc.vector.tensor_copy(kt[:], pp[:])

        def produce_v_half(g, half):
            xt = xT_g[g]
            if half == 0:
                vs = vsp.tile([P, GT * 130], BF16, tag="vs")
                v_g[g] = vs
            else:
                vs = v_g[g]
            for j in (0, 1) if half == 0 else (2, 3):
                pv = aux.tile([P, P], FP32, tag="aux")
                for c in range(DC):
                    nc.tensor.matmul(
                        pv[:],
                        xt[:, c * SBW + j * P : c * SBW + (j + 1) * P],
                        wv_b[:, c * P : (c + 1) * P],
                        start=(c == 0),
                        stop=(c == DC - 1),
                    )
                dstv = vs[:, j * 130 : j * 130 + 130].rearrange(
                    "p (h q) -> p h q", h=2
                )[:, :, 0:64]
                nc.vector.tensor_copy(
                    dstv, pv[:].rearrange("p (h q) -> p h q", h=2)
                )
            if half == 1:
                ones_cols = vs[:].rearrange("p (t q) -> p t q", t=GT)[
                    :, :, 64:130:65
                ]
                nc.vector.memset(ones_cols, 1.0)

        def produce_group(g):
            produce_x_q(g)
            produce_k(g)
            produce_v_half(g, 0)
            produce_v_half(g, 1)

        # ---- attention + output projection ----
        DEFER_ITERS = 12
        pending = [None]

        def epilogue_part1(sb, po0, po1):
            # osb is the unnormalized [h0|h1, s] concat; the softmax
            # denominators live in row 64 of each po bank (ones column).
            osb = opool.tile([P, SBW], BF16, tag="osb")
            nc.vector.tensor_copy(osb[0:64, :], po0[0:64, :])
            nc.vector.tensor_copy(osb[64:128, :], po1[0:64, :])
            s0 = spool.tile([1, SBW], FP32, tag="s0")
            s1 = spool.tile([1, SBW], FP32, tag="s1")
            nc.vector.tensor_copy(s0[:], po0[64:65, :])
            nc.vector.tensor_copy(s1[:], po1[64:65, :])
            nc.sync.dma_start(zscr_d[sb, 0:1], s0[:])
            nc.sync.dma_start(zscr_d[sb, 1:2], s1[:])
            # gather Z back transposed and take reciprocals now: the DMA
            # round-trip latency hides in the 12-iter defer gap instead of
            # stalling the projection pipeline in part 2
            zT = spool.tile([P, 2 * GT], FP32, tag="zT")
            for hh in range(2):
                nc.sync.dma_start(
                    zT[:, hh * GT : (hh + 1) * GT],
                    zscr_d[sb, hh].rearrange("(st p) -> p st", p=P),
                )
            rc = spool.tile([P, 2 * GT], FP32, tag="rc")
            nc.vector.reciprocal(rc[:], zT[:])
            pending[0] = (sb, osb, rc)

        def epilogue_part2():
            if pending[0] is None:
                return
            sb, osb, rc = pending[0]
            pending[0] = None
            for st in range(GT):
                sl = slice(st * P, (st + 1) * P)
                py0 = aux.tile([P, D], FP32, tag="aux")
                py1 = aux.tile([P, D], FP32, tag="aux")
                nc.tensor.matmul(
                    py0[:], osb[0:64, sl], wo_b[0:64, :],
                    start=True, stop=True, tile_position=(0, 0),
                )
                nc.tensor.matmul(
                    py1[:], osb[64:128, sl], wo_b[64:128, :],
                    start=True, stop=True, tile_position=(64, 0),
                )
                t0 = ypool.tile([P, D], FP32, tag="t0")
                # run the h0 scale on ScalarE (per-partition scale AP): it is
                # half-idle, while VectorE's queue sits behind 1.2us fast-exp
                # ops and would stall the next projection pair on the aux WAR
                nc.scalar.activation(
                    t0[:], py0[:], mybir.ActivationFunctionType.Copy,
                    bias=0.0, scale=rc[:, st : st + 1],
                )
                t1 = ypool.tile([P, D], FP32, tag="t1")
                nc.scalar.activation(
                    t1[:], py1[:], mybir.ActivationFunctionType.Copy,
                    bias=0.0, scale=rc[:, GT + st : GT + st + 1],
                )
                ysb = ypool.tile([P, D], FP32, tag="y")
                nc.vector.tensor_tensor(
                    ysb[:], t0[:], t1[:], op=mybir.AluOpType.add
                )
                row = (sb * GT + st) * P
                nc.sync.dma_start(y_d[row : row + P, :], ysb[:])

        produce_group(0)
        produce_group(1)

        PREF = 8
        SPLICE = {}
        for _g in range(2, NSB):
            base = 2 + (_g - 2) * 4
            SPLICE[base] = lambda g=_g: produce_x_q(g)
            SPLICE[base + 1] = lambda g=_g: produce_k(g)
            SPLICE[base + 2] = lambda g=_g: produce_v_half(g, 0)
            SPLICE[base + 3] = lambda g=_g: produce_v_half(g, 1)
        NQ = NSB * NT
        po_cur = [None, None]  # po bank, psZ bank
        ex_q = {}
        for q in range(NQ + PREF):
            if q < NQ:
                sb, tt = q // NT, q % NT
                g, j = tt // GT, tt % GT
                if sb == 0 and tt in SPLICE:
                    SPLICE[tt]()
                kt, qt = kT_g[g], qT_g[sb]
                ps_t = psp.tile([P, 2 * SBW], FP32, tag="ps")
                nc.tensor.matmul(
                    ps_t[:, 0:SBW],
                    kt[0:64, j * P : (j + 1) * P],
                    qt[0:64, :],
                    start=True,
                    stop=True,
                    tile_position=(0, 0),
                )
                nc.tensor.matmul(
                    ps_t[:, SBW : 2 * SBW],
                    kt[64:128, j * P : (j + 1) * P],
                    qt[64:128, :],
                    start=True,
                    stop=True,
                    tile_position=(64, 0),
                )
                # Alternate whole iterations between the two exp engines:
                # shared-tile writes from two engines serialize under Tile's
                # hazard tracking, but disjoint iterations run in parallel.
                ex = epool.tile([P, 2 * SBW], BF16, tag="exp")
                if q % 2 == 0:
                    nc.scalar.activation(
                        ex[:], ps_t[:],
                        mybir.ActivationFunctionType.Exp,
                        scale=float(1.0 / np.sqrt(DR)),
                    )
                else:
                    nc.vector.tensor_scalar(
                        ex[:].bitcast(I16),
                        ps_t[:],
                        A16,
                        B16,
                        op0=mybir.AluOpType.mult,
                        op1=mybir.AluOpType.add,
                    )
                ex_q[q] = ex
                if tt == DEFER_ITERS:
                    epilogue_part2()
            if q >= PREF:
                qa = q - PREF
                sba, ta = qa // NT, qa % NT
                ga, ja = ta // GT, ta % GT
                if ta == 0:
                    po_cur[0] = pop.tile([65, SBW], FP32, tag="po", name="po0")
                    po_cur[1] = pop.tile([65, SBW], FP32, tag="po", name="po1")
                po0, po1 = po_cur
                vs, ex = v_g[ga], ex_q.pop(qa)
                nc.tensor.matmul(
                    po0[:],
                    vs[:, ja * 130 : ja * 130 + 65],
                    ex[:, 0:SBW],
                    start=(ta == 0),
                    stop=(ta == NT - 1),
                )
                nc.tensor.matmul(
                    po1[:],
                    vs[:, ja * 130 + 65 : ja * 130 + 130],
                    ex[:, SBW : 2 * SBW],
                    start=(ta == 0),
                    stop=(ta == NT - 1),
                )
                if ta == NT - 1:
                    epilogue_part1(sba, po0, po1)
        epilogue_part2()

    _split_sync_waits(nc)
    return nc


_program = None


def _get_program():
    global _program
    if _program is None:
        _program = _build_program()
    return _program


def _make_in_maps(x, Wq, Wk, Wv, Wo):
    import ml_dtypes

    f16 = ml_dtypes.bfloat16
    xts = [np.ascontiguousarray(x[b].T).astype(f16) for b in range(B)]
    in_maps = []
    for c in range(N_CORES):
        b = c // 4
        hp = c % 4
        h0, h1 = 2 * hp, 2 * hp + 1
        in_maps.append(
            {
                "xt": xts[b],
                "wq": np.ascontiguousarray(
                    np.concatenate([Wq[h0], Wq[h1]], axis=1)
                ).astype(f16),
                "wk": np.ascontiguousarray(
                    np.concatenate([Wk[h0], Wk[h1]], axis=1)
                ).astype(f16),
                "wv": np.ascontiguousarray(
                    np.concatenate([Wv[h0], Wv[h1]], axis=1)
                ).astype(f16),
                "wo": np.ascontiguousarray(Wo[hp * 128 : (hp + 1) * 128]).astype(
                    f16
                ),
            }
        )
    return in_maps


def kernel(**inputs) -> np.ndarray:
    x = np.asarray(inputs["x"], dtype=np.float32)
    Wq = np.asarray(inputs["Wq"], dtype=np.float32)
    Wk = np.asarray(inputs["Wk"], dtype=np.float32)
    Wv = np.asarray(inputs["Wv"], dtype=np.float32)
    Wo = np.asarray(inputs["Wo"], dtype=np.float32)
    bo = np.asarray(inputs["bo"], dtype=np.float32)

    nc = _get_program()
    in_maps = _make_in_maps(x, Wq, Wk, Wv, Wo)
    res = run_bass_kernel_spmd(nc, in_maps, list(range(N_CORES)))

    y = np.zeros((B, S, D), dtype=np.float32)
    for c in range(N_CORES):
        y[c // 4] += np.asarray(res.results[c]["y"], dtype=np.float32)
    y += bo[None, None, :]
    return y


# revision 46
# speedup vs baseline: 1.0260x; 1.0260x over previous
"""Multi-head attention (B=2, S=4096, D=512, H=8, DR=64) on 8 trn2 NeuronCores.

Sharding: core c -> batch b = c // 4, head-pair hp = c % 4 (heads 2*hp, 2*hp+1).
Each core computes q/k/v projections, flash-style attention with scores kept
in transposed [t, s] orientation, and the partial output projection for its
two heads. Host sums the 4 partials per batch and adds the bias.

Pipeline notes (all on-chip data bf16; fp32 only in PSUM accumulators):
- exp alternates whole iterations between ScalarE (exact exp) and VectorE
  (Schraudolph fast-exp: int16(x*A+B) bit-cast as bf16). Alternating whole
  iterations (rather than splitting columns of one tile) keeps the two
  engines' writes on disjoint tiles, so Tile's hazard tracking lets them
  run concurrently; the serial exp stream was the original bottleneck.
  The softmax denominator sums the same quantized weights, so attention
  rows still sum to 1; the fast-exp error averages out over 4096-key sums.
- AV keeps the ones-column trick (M=65 per head): softmax denominators
  accumulate for free in row 64 of each po bank.
- Normalization is deferred past the output projection: y = py0*r0 + py1*r1
  with per-partition reciprocals obtained by round-tripping the Z rows
  through DRAM (DMA moves data across partitions; engines cannot).
"""

import sys

for _p in ("/opt/trn_rl_repo", "/root/.axon_site/_ro/trn_rl_repo"):
    if _p not in sys.path:
        sys.path.insert(0, _p)

import os as _osmod
import numpy as np
from contextlib import ExitStack

KDBG = _osmod.environ.get("KDBG", "")

import concourse.bass as bass
import concourse.tile as tile
import concourse.mybir as mybir
from concourse.bass_utils import run_bass_kernel_spmd
from concourse.masks import make_identity

B, S, D = 2, 4096, 512
H, DR = 8, 64
P = 128
NT = S // P          # 32 t-tiles (also s-tiles)
SBW = 512            # s-block width
NSB = S // SBW       # 8 s-blocks / t-groups
DC = D // P          # 4 d-chunks
GT = SBW // P        # 4 t-tiles per group
N_CORES = 8
FP32 = mybir.dt.float32
BF16 = mybir.dt.bfloat16
I16 = mybir.dt.int16

# exp split and fast-exp constants (bf16 bit-trick domain)
CEXP = 352           # ScalarE handles cols [0:CEXP) per head per 512-block
LOG2E = 1.4426950408889634
A16 = 128.0 * LOG2E / 8.0           # applied to raw (unscaled) scores
B16 = 128.0 * (127.0 - 0.0861)


# This repo's walrus invocation hardcodes --enable-ldw-opt=false, which keeps
# every LDWEIGHTS serialized with its MATMUL. walrus' LDW optimization rejects
# standalone InstLdweights, so fuse each one into its following Matmult
# (ldweights=true) in the BIR json, then flip the flag on the command line.
def _fuse_ldweights_json(path_in, path_out):
    import json as _json

    d = _json.load(open(path_in))
    n_noop = [0]
    for f in d.get("functions", []):
        for b in f.get("blocks", []):
            insts = b.get("instructions", [])
            out = []
            pend = None
            for i in insts:
                if i.get("engine") != "PE":
                    out.append(i)
                    continue
                if i["opcode"] == "Ldweights":
                    assert pend is None
                    pend = i
                    continue
                if i["opcode"] == "Matmult" and pend is not None:
                    assert pend["ins"][0] == i["ins"][1], (
                        f"weights AP mismatch {pend['name']} vs {i['name']}"
                    )
                    i["ldweights"] = True
                    sl = pend.get("sync_info") or {}
                    sm = i.get("sync_info") or {}
                    waits = list(sl.get("on_wait") or []) + list(
                        sm.get("on_wait") or []
                    )
                    upds = list(sl.get("on_update") or []) + list(
                        sm.get("on_update") or []
                    )
                    while len(waits) > 1:
                        w = waits.pop(0)
                        n_noop[0] += 1
                        out.append(
                            {
                                "name": f"I-fusenoop-{n_noop[0]}",
                                "engine": "PE",
                                "opcode": "NoOp",
                                "debug": i.get("debug", 0),
                                "ins": [],
                                "outs": [],
                                "sync_info": {"on_wait": [w], "on_update": []},
                            }
                        )
                    i["sync_info"] = {"on_wait": waits, "on_update": upds}
                    pend = None
                    out.append(i)
                    continue
                out.append(i)
            assert pend is None, "trailing Ldweights"
            b["instructions"] = out
    _json.dump(d, open(path_out, "w"))


def _patch_ldw_opt():
    import os as _os
    from concourse import bass_utils as _bu

    if getattr(_bu, "_ldw_opt_patched", False):
        return
    _bu._ldw_opt_patched = True
    _orig = _bu.run_command

    def patched_run(argv, **kwargs):
        return _orig(argv, **kwargs)

    _bu.run_command = patched_run


_patch_ldw_opt()

_drain_patched = False


def _patch_tile_drain():
    """This walrus build rejects >1 sync wait on one instruction, which breaks
    TileContext's kernel-tail drain. Spread the waits over nop instructions
    emitted just before the drain."""
    global _drain_patched
    if _drain_patched:
        return
    _drain_patched = True

    def patched(self, tick_clock, wait_clock):
        nop0 = self.nc.sync.nop()
        wait_clock.add_sem_waits(
            nop0.ins, tile.ScopedClock({None: tick_clock.global_clock})
        )
        si = nop0.ins.sync_info
        waits = list(si.on_wait) if si is not None else []
        if waits:
            nop0.ins.sync_info = mybir.SyncInfo(on_wait=waits[:1], on_update=[])
            for w in waits[1:]:
                nop = self.nc.sync.nop()
                nop.ins.sync_info = mybir.SyncInfo(on_wait=[w], on_update=[])
        self.nc.sync.drain()
        self.nc.all_engine_barrier()
        popped = self.nc._tile_sem_poison_stack.pop()
        assert popped is self._sem_poison
        self.nc.clear_and_free_semaphores(list(self.sems.allocated().values()))
        self.nc.all_engine_barrier()

    tile.TileContext._drain_and_barrier = patched


# This walrus build supports only one sync-wait slot per instruction, while
# Tile's sem-assigner attaches up to ~3. Spread the excess onto NoOp
# instructions inserted immediately before the owning instruction.
_WAIT_LIMIT = 1
_SKIP_OPCODES = {"AllEngineBarrier", "EventSemaphore", "Call"}


def _split_sync_waits(nc: bass.Bass):
    noop_cls = getattr(mybir, "InstNoOp", None)
    if noop_cls is None:
        import bass_rust

        noop_cls = bass_rust.InstNoOp
    counter = [0]
    for f in nc.m.functions:
        for blk in f.blocks:
            insts = blk.instructions
            new_list = []
            changed = False
            for inst in insts:
                si = inst.sync_info
                waits = list(si.on_wait) if si is not None and si.on_wait else []
                wait_limit = 0 if inst.opcode == "Ldweights" else _WAIT_LIMIT
                if (
                    len(waits) > wait_limit
                    and inst.opcode not in _SKIP_OPCODES
                    and all(w.sync_type == "semaphore" for w in waits)
                ):
                    excess = waits[: len(waits) - wait_limit]
                    keep = waits[len(waits) - wait_limit :]
                    for w in excess:
                        counter[0] += 1
                        new_list.append(
                            noop_cls(
                                name=f"I-waitsplit-{counter[0]}",
                                engine=inst.engine,
                                debug=inst.debug,
                                ins=[],
                                outs=[],
                                sync_info=mybir.SyncInfo(
                                    on_wait=[w], on_update=[]
                                ),
                            )
                        )
                    inst.sync_info = mybir.SyncInfo(
                        on_wait=keep, on_update=list(si.on_update or [])
                    )
                    changed = True
                new_list.append(inst)
            if changed:
                insts.clear()
                insts.extend(new_list)


def _build_program() -> bass.Bass:
    _patch_tile_drain()
    nc = bass.Bass()

    xt_d = nc.declare_dram_parameter("xt", [D, S], BF16, isOutput=False)
    wq_d = nc.declare_dram_parameter("wq", [D, P], BF16, isOutput=False)
    wk_d = nc.declare_dram_parameter("wk", [D, P], BF16, isOutput=False)
    wv_d = nc.declare_dram_parameter("wv", [D, P], BF16, isOutput=False)
    wo_d = nc.declare_dram_parameter("wo", [P, D], BF16, isOutput=False)
    y_d = nc.declare_dram_parameter("y", [S, D], FP32, isOutput=True)
    zscr_d = nc.declare_dram_parameter("zscr", [NSB, 2, SBW], FP32, isOutput=True)

    with tile.TileContext(nc) as tc, ExitStack() as ctx:
        consts = ctx.enter_context(tc.tile_pool(name="consts", bufs=1))
        wpool = ctx.enter_context(tc.tile_pool(name="weights", bufs=1))
        aux = ctx.enter_context(tc.tile_pool(name="aux", bufs=2, space="PSUM"))
        psp = ctx.enter_context(tc.tile_pool(name="ps", bufs=2, space="PSUM"))
        pop = ctx.enter_context(tc.tile_pool(name="po", bufs=2, space="PSUM"))
        epool = ctx.enter_context(tc.tile_pool(name="exp", bufs=12))
        spool = ctx.enter_context(tc.tile_pool(name="small", bufs=4))
        opool = ctx.enter_context(tc.tile_pool(name="osb", bufs=2))
        ypool = ctx.enter_context(tc.tile_pool(name="yout", bufs=3))


        # PE warm-up: junk matmuls during the initial DMA window keep the HAM
        # clock-gate at 8/8 so the first real matmuls run at 2.4 GHz.
        warm = consts.tile([P, D], BF16)
        nc.vector.memset(warm[:], 0.0)
        pw = aux.tile([P, D], FP32, tag="aux", name="pw")
        for _ in range(10):
            nc.tensor.matmul(
                pw[:], warm[:, 0:P], warm[:], start=True, stop=True
            )
        if "selprobe" in KDBG:
            pdbg = aux.tile([P, 16], FP32, tag="aux", name="pdbg")
            nc.tensor.matmul(
                pdbg[:, 0:4], warm[:, 0:P], zsel[:], start=True, stop=True
            )
            sdbg = spool.tile([P, 4], FP32, tag="sdbg")
            nc.vector.tensor_copy(sdbg[:], pdbg[:, 0:4])

        # Weights fp16; w*_b[p, c*128 + e] = W[c*128 + p, e]
        wq_b = wpool.tile([P, D], BF16)
        wk_b = wpool.tile([P, D], BF16)
        wv_b = wpool.tile([P, D], BF16)
        wo_b = wpool.tile([P, D], BF16)
        for w_b, w_dram in ((wq_b, wq_d), (wk_b, wk_d), (wv_b, wv_d)):
            nc.gpsimd.dma_start(
                w_b[:].rearrange("p (c e) -> p c e", c=DC),
                w_dram[:].rearrange("(c p) e -> p c e", p=P),
            )
        nc.gpsimd.dma_start(wo_b[:], wo_d[:])

        # Per-group persistent tiles.
        # xT_g[g][p, c*512 + j] = x[g*512 + j, c*128 + p]
        xtp = ctx.enter_context(tc.tile_pool(name="xtg", bufs=NSB))
        ktp = ctx.enter_context(tc.tile_pool(name="ktg", bufs=NSB))
        qtp = ctx.enter_context(tc.tile_pool(name="qtg", bufs=NSB))
        vsp = ctx.enter_context(tc.tile_pool(name="vsg", bufs=NSB))
        xT_g = [None] * NSB
        kT_g = [None] * NSB   # [e(h0|h1), 512 t-cols]
        qT_g = [None] * NSB   # [e(h0|h1), 512 s-cols]
        v_g = [None] * NSB    # [t, 4 t-tiles * (h0 64 | h1 64)]

        def produce_x_q(g):
            xt = xtp.tile([P, DC * SBW], BF16, tag="xt")
            xT_g[g] = xt
            for c in range(DC):
                nc.sync.dma_start(
                    xt[:, c * SBW : (c + 1) * SBW],
                    xt_d[c * P : (c + 1) * P, g * SBW : (g + 1) * SBW],
                )
            qt = qtp.tile([P, SBW], BF16, tag="qt")
            qT_g[g] = qt
            pp = aux.tile([P, SBW], FP32, tag="aux")
            for c in range(DC):
                nc.tensor.matmul(
                    pp[:],
                    wq_b[:, c * P : (c + 1) * P],
                    xt[:, c * SBW : (c + 1) * SBW],
                    start=(c == 0),
                    stop=(c == DC - 1),
                )
            nc.vector.tensor_copy(qt[:], pp[:])

        def produce_k(g):
            xt = xT_g[g]
            kt = ktp.tile([P, SBW], BF16, tag="kt")
            kT_g[g] = kt
            pp = aux.tile([P, SBW], FP32, tag="aux")
            for c in range(DC):
                nc.tensor.matmul(
                    pp[:],
                    wk_b[:, c * P : (c + 1) * P],
                    xt[:, c * SBW : (c + 1) * SBW],
                    start=(c == 0),
                    stop=(c == DC - 1),
                )
            nc.vector.tensor_copy(kt[:], pp[:])

        def produce_v_half(g, half):
            xt = xT_g[g]
            if half == 0:
                vs = vsp.tile([P, GT * 130], BF16, tag="vs")
                v_g[g] = vs
            else:
                vs = v_g[g]
            for j in (0, 1) if half == 0 else (2, 3):
                pv = aux.tile([P, P], FP32, tag="aux")
                for c in range(DC):
                    nc.tensor.matmul(
                        pv[:],
                        xt[:, c * SBW + j * P : c * SBW + (j + 1) * P],
                        wv_b[:, c * P : (c + 1) * P],
                        start=(c == 0),
                        stop=(c == DC - 1),
                    )
                dstv = vs[:, j * 130 : j * 130 + 130].rearrange(
                    "p (h q) -> p h q", h=2
                )[:, :, 0:64]
                nc.vector.tensor_copy(
                    dstv, pv[:].rearrange("p (h q) -> p h q", h=2)
                )
            if half == 1:
                ones_cols = vs[:].rearrange("p (t q) -> p t q", t=GT)[
                    :, :, 64:130:65
                ]
                nc.vector.memset(ones_cols, 1.0)

        def produce_group(g):
            produce_x_q(g)
            produce_k(g)
            produce_v_half(g, 0)
            produce_v_half(g, 1)

        # ---- attention + output projection ----
        DEFER_ITERS = 12
        pending = [None]

        def epilogue_part1(sb, po0, po1):
            # osb is the unnormalized [h0|h1, s] concat; the softmax
            # denominators live in row 64 of each po bank (ones column).
            osb = opool.tile([P, SBW], BF16, tag="osb")
            nc.vector.tensor_copy(osb[0:64, :], po0[0:64, :])
            nc.vector.tensor_copy(osb[64:128, :], po1[0:64, :])
            s0 = spool.tile([1, SBW], FP32, tag="s0")
            s1 = spool.tile([1, SBW], FP32, tag="s1")
            nc.vector.tensor_copy(s0[:], po0[64:65, :])
            nc.vector.tensor_copy(s1[:], po1[64:65, :])
            nc.sync.dma_start(zscr_d[sb, 0:1], s0[:])
            nc.sync.dma_start(zscr_d[sb, 1:2], s1[:])
            # gather Z back transposed and take reciprocals now: the DMA
            # round-trip latency hides in the 12-iter defer gap instead of
            # stalling the projection pipeline in part 2
            zT = spool.tile([P, 2 * GT], FP32, tag="zT")
            for hh in range(2):
                nc.sync.dma_start(
                    zT[:, hh * GT : (hh + 1) * GT],
                    zscr_d[sb, hh].rearrange("(st p) -> p st", p=P),
                )
            rc = spool.tile([P, 2 * GT], FP32, tag="rc")
            nc.vector.reciprocal(rc[:], zT[:])
            pending[0] = (sb, osb, rc)

        def epilogue_part2():
            if pending[0] is None:
                return
            sb, osb, rc = pending[0]
            pending[0] = None
            for st in range(GT):
                sl = slice(st * P, (st + 1) * P)
                py0 = aux.tile([P, D], FP32, tag="aux")
                py1 = aux.tile([P, D], FP32, tag="aux")
                nc.tensor.matmul(
                    py0[:], osb[0:64, sl], wo_b[0:64, :],
                    start=True, stop=True, tile_position=(0, 0),
                )
                nc.tensor.matmul(
                    py1[:], osb[64:128, sl], wo_b[64:128, :],
                    start=True, stop=True, tile_position=(64, 0),
                )
                t0 = ypool.tile([P, D], FP32, tag="t0")
                # run the h0 scale on ScalarE (per-partition scale AP): it is
                # half-idle, while VectorE's queue sits behind 1.2us fast-exp
                # ops and would stall the next projection pair on the aux WAR
                nc.scalar.activation(
                    t0[:], py0[:], mybir.ActivationFunctionType.Copy,
                    bias=0.0, scale=rc[:, st : st + 1],
                )
                ysb = ypool.tile([P, D], FP32, tag="y")
                nc.vector.scalar_tensor_tensor(
                    ysb[:], py1[:], rc[:, GT + st : GT + st + 1], t0[:],
                    op0=mybir.AluOpType.mult, op1=mybir.AluOpType.add,
                )
                row = (sb * GT + st) * P
                nc.sync.dma_start(y_d[row : row + P, :], ysb[:])

        produce_group(0)
        produce_group(1)

        PREF = 8
        SPLICE = {}
        for _g in range(2, NSB):
            base = 2 + (_g - 2) * 4
            SPLICE[base] = lambda g=_g: produce_x_q(g)
            SPLICE[base + 1] = lambda g=_g: produce_k(g)
            SPLICE[base + 2] = lambda g=_g: produce_v_half(g, 0)
            SPLICE[base + 3] = lambda g=_g: produce_v_half(g, 1)
        NQ = NSB * NT
        po_cur = [None, None]  # po bank, psZ bank
        ex_q = {}
        for q in range(NQ + PREF):
            if q < NQ:
                sb, tt = q // NT, q % NT
                g, j = tt // GT, tt % GT
                if sb == 0 and tt in SPLICE:
                    SPLICE[tt]()
                kt, qt = kT_g[g], qT_g[sb]
                ps_t = psp.tile([P, 2 * SBW], FP32, tag="ps")
                nc.tensor.matmul(
                    ps_t[:, 0:SBW],
                    kt[0:64, j * P : (j + 1) * P],
                    qt[0:64, :],
                    start=True,
                    stop=True,
                    tile_position=(0, 0),
                )
                nc.tensor.matmul(
                    ps_t[:, SBW : 2 * SBW],
                    kt[64:128, j * P : (j + 1) * P],
                    qt[64:128, :],
                    start=True,
                    stop=True,
                    tile_position=(64, 0),
                )
                # Alternate whole iterations between the two exp engines:
                # shared-tile writes from two engines serialize under Tile's
                # hazard tracking, but disjoint iterations run in parallel.
                ex = epool.tile([P, 2 * SBW], BF16, tag="exp")
                if q % 2 == 0:
                    nc.scalar.activation(
                        ex[:], ps_t[:],
                        mybir.ActivationFunctionType.Exp,
                        scale=float(1.0 / np.sqrt(DR)),
                    )
                else:
                    nc.vector.tensor_scalar(
                        ex[:].bitcast(I16),
                        ps_t[:],
                        A16,
                        B16,
                        op0=mybir.AluOpType.mult,
                        op1=mybir.AluOpType.add,
                    )
                ex_q[q] = ex
                if tt == DEFER_ITERS:
                    epilogue_part2()
            if q >= PREF:
                qa = q - PREF
                sba, ta = qa // NT, qa % NT
                ga, ja = ta // GT, ta % GT
                if ta == 0:
                    po_cur[0] = pop.tile([65, SBW], FP32, tag="po", name="po0")
                    po_cur[1] = pop.tile([65, SBW], FP32, tag="po", name="po1")
                po0, po1 = po_cur
                vs, ex = v_g[ga], ex_q.pop(qa)
                nc.tensor.matmul(
                    po0[:],
                    vs[:, ja * 130 : ja * 130 + 65],
                    ex[:, 0:SBW],
                    start=(ta == 0),
                    stop=(ta == NT - 1),
                )
                nc.tensor.matmul(
                    po1[:],
                    vs[:, ja * 130 + 65 : ja * 130 + 130],
                    ex[:, SBW : 2 * SBW],
                    start=(ta == 0),
                    stop=(ta == NT - 1),
                )
                if ta == NT - 1:
                    epilogue_part1(sba, po0, po1)
        epilogue_part2()

    _split_sync_waits(nc)
    return nc


_program = None


def _get_program():
    global _program
    if _program is None:
        _program = _build_program()
    return _program


def _make_in_maps(x, Wq, Wk, Wv, Wo):
    import ml_dtypes

    f16 = ml_dtypes.bfloat16
    xts = [np.ascontiguousarray(x[b].T).astype(f16) for b in range(B)]
    in_maps = []
    for c in range(N_CORES):
        b = c // 4
        hp = c % 4
        h0, h1 = 2 * hp, 2 * hp + 1
        in_maps.append(
            {
                "xt": xts[b],
                "wq": np.ascontiguousarray(
                    np.concatenate([Wq[h0], Wq[h1]], axis=1)
                ).astype(f16),
                "wk": np.ascontiguousarray(
                    np.concatenate([Wk[h0], Wk[h1]], axis=1)
                ).astype(f16),
                "wv": np.ascontiguousarray(
                    np.concatenate([Wv[h0], Wv[h1]], axis=1)
                ).astype(f16),
                "wo": np.ascontiguousarray(Wo[hp * 128 : (hp + 1) * 128]).astype(
                    f16
                ),
            }
        )
    return in_maps


def kernel(**inputs) -> np.ndarray:
    x = np.asarray(inputs["x"], dtype=np.float32)
    Wq = np.asarray(inputs["Wq"], dtype=np.float32)
    Wk = np.asarray(inputs["Wk"], dtype=np.float32)
    Wv = np.asarray(inputs["Wv"], dtype=np.float32)
    Wo = np.asarray(inputs["Wo"], dtype=np.float32)
    bo = np.asarray(inputs["bo"], dtype=np.float32)

    nc = _get_program()
    in_maps = _make_in_maps(x, Wq, Wk, Wv, Wo)
    res = run_bass_kernel_spmd(nc, in_maps, list(range(N_CORES)))

    y = np.zeros((B, S, D), dtype=np.float32)
    for c in range(N_CORES):
        y[c // 4] += np.asarray(res.results[c]["y"], dtype=np.float32)
    y += bo[None, None, :]
    return y
